# revision 7
# baseline (speedup 1.0000x reference)
"""AdaQLinear (W8A8 dynamic-act-quant linear) on 8 TRN2 NeuronCores.

  reference:
    x_scale = max(|x|)/127                      (global, dynamic)
    qx = clip(round(x/x_scale), -128, 127)      int8
    acc = qx @ qw.T                             int32
    y_i8 = clip(round(acc*alpha) + b_i8, -128, 127)
    y = (y_i8 * y_scale).fp16

  Sharding: data-parallel over tokens (8192/8 = 1024 per core); qw/b/scales
  replicated.  Each core needs the *global* absmax of x, obtained with a
  tiny AllReduce(max) of per-partition maxima.

  The int8 GEMM runs as bf16 matmul: qx,qw in [-128,127] are exact in bf16
  and PSUM accumulates fp32 (sums stay far below 2^24), so results match
  the int32 reference bit-exactly.  Requant = one ACT op per tile
  (rint(acc*alpha + b) with saturating int8 cast == round+add+clip since b
  is integral) + one DVE scale-mul to fp16.
"""
import os
import numpy as np
import ml_dtypes

import concourse.bass as bass
import concourse.bass_isa as bass_isa
import concourse.tile as tile
from concourse import bacc, mybir
import concourse.bass_utils as bass_utils
from concourse.bass_utils import run_bass_kernel_spmd

# Don't ship trace artifacts anywhere external; keep them local.
bass_utils.upload_artifacts = lambda tmpdir: ""


def _ensure_ntff_hook():
    """Register the axon NTFF profiling hook if the image's antenv lacks
    the axon_hooks shim (needed for exec_time_ns with trace=True)."""
    try:
        from antenv.axon_hooks import get_axon_ntff_profile_hook  # noqa: F401
        return
    except ImportError:
        pass
    import sys
    import types
    hook = None
    try:
        from trn_agent_boot.trn_boot import _ntff_profile_via_ctypes
        so = "/opt/axon/libaxon_pjrt.so"
        if os.path.exists(so):
            hook = _ntff_profile_via_ctypes(so)
    except Exception:
        hook = None
    mod = types.ModuleType("antenv.axon_hooks")
    mod._hook = hook
    mod.get_axon_ntff_profile_hook = lambda: mod._hook
    mod.set_axon_ntff_profile_hook = lambda h: setattr(mod, "_hook", h)
    sys.modules["antenv.axon_hooks"] = mod
    try:
        import antenv
        antenv.axon_hooks = mod
    except Exception:
        pass

N_TOK, D_IN, D_OUT = 8192, 4096, 4096
N_CORES = 8
N_SH = N_TOK // N_CORES      # 1024 tokens per core
KT = D_IN // 128             # 32 contraction tiles
MT = D_OUT // 128            # 32 output-row tiles
NH = N_SH // 512             # 2 psum-width halves

F16, BF16, F32 = mybir.dt.float16, mybir.dt.bfloat16, mybir.dt.float32
I8 = mybir.dt.int8

_nc_cache = {}
LAST_EXEC_NS = None


def _build():
    nc = bacc.Bacc("TRN2", target_bir_lowering=False, debug=False,
                   num_devices=N_CORES)
    xT_d = nc.dram_tensor("xT", [D_IN, N_SH], F16, kind="ExternalInput")
    qw_d = nc.dram_tensor("qwt", [MT, 128, KT, 128], BF16, kind="ExternalInput")
    b_d = nc.dram_tensor("bt", [128, MT], I8, kind="ExternalInput")
    al_d = nc.dram_tensor("alpha_b", [128, 1], F32, kind="ExternalInput")
    ys_d = nc.dram_tensor("ys_b", [128, 1], F32, kind="ExternalInput")
    out_d = nc.dram_tensor("out", [D_OUT, N_SH], F16, kind="ExternalOutput")
    cc_out = nc.dram_tensor("cc_out", [128, 1], F32, addr_space="Shared")

    with tile.TileContext(nc) as tc:
        with tc.tile_pool(name="const", bufs=1) as cp, \
             tc.tile_pool(name="xin", bufs=1) as xp, \
             tc.tile_pool(name="qx", bufs=1) as qxp, \
             tc.tile_pool(name="qi", bufs=4) as qip, \
             tc.tile_pool(name="ax", bufs=2) as axp, \
             tc.tile_pool(name="qw", bufs=5) as qwp, \
             tc.tile_pool(name="ps", bufs=6, space="PSUM") as psp, \
             tc.tile_pool(name="yi", bufs=4) as yip, \
             tc.tile_pool(name="yo", bufs=3) as yop, \
             tc.tile_pool(name="dram", bufs=1, space="DRAM") as dp:

            # ---- constants
            b_i8 = cp.tile([128, MT], I8, tag="bi8")
            nc.sync.dma_start(out=b_i8[:], in_=b_d[:])
            b_f32 = cp.tile([128, MT], F32, tag="bf32")
            nc.vector.tensor_copy(b_f32[:], b_i8[:])
            al_sb = cp.tile([128, 1], F32, tag="al")
            nc.sync.dma_start(out=al_sb[:], in_=al_d[:])
            ys_sb = cp.tile([128, 1], F32, tag="ys")
            nc.sync.dma_start(out=ys_sb[:], in_=ys_d[:])

            # ---- load x shard (2kt chunks); absmax (u16-mask abs, TT max)
            CH = 2                      # kt per chunk
            xch = []
            acc = cp.tile([128, CH * N_SH], F16, tag="amacc")
            nc.vector.memset(acc[:], 0.0)
            for c in range(KT // CH):
                xt = xp.tile([128, CH, N_SH], F16, tag=f"x{c}")
                nc.sync.dma_start(
                    out=xt[:],
                    in_=xT_d[c * CH * 128:(c + 1) * CH * 128, :].rearrange(
                        "(k p) n -> p k n", p=128))
                xch.append(xt)
                ax = axp.tile([128, CH * N_SH], F16, tag="ax")
                nc.vector.tensor_scalar(ax[:].bitcast(mybir.dt.uint16),
                                        xt[:].opt().bitcast(mybir.dt.uint16),
                                        0x7FFF, None,
                                        op0=mybir.AluOpType.bitwise_and)
                nc.vector.tensor_tensor(acc[:], acc[:], ax[:],
                                        op=mybir.AluOpType.max)
            am = cp.tile([128, 1], F32, tag="am")
            nc.vector.tensor_reduce(am[:], acc[:], axis=mybir.AxisListType.X,
                                    op=mybir.AluOpType.max,
                                    apply_absolute_value=True)

            # ---- global absmax: AllReduce(max) then cross-partition max
            cc_in = dp.tile([128, 1], F32)
            nc.sync.dma_start(out=cc_in[:], in_=am[:])
            nc.gpsimd.collective_compute(
                "AllReduce", mybir.AluOpType.max,
                replica_groups=[list(range(N_CORES))],
                ins=[cc_in[:].opt()], outs=[cc_out.ap().opt()])
            gm_in = cp.tile([128, 1], F32, tag="gmin")
            nc.sync.dma_start(out=gm_in[:], in_=cc_out[:, :])
            gm = cp.tile([128, 1], F32, tag="gm")
            nc.gpsimd.partition_all_reduce(gm[:], gm_in[:], 128,
                                           bass_isa.ReduceOp.max)
            inv = cp.tile([128, 1], F32, tag="inv")
            nc.vector.reciprocal(inv[:], gm[:])
            nc.vector.tensor_scalar(inv[:], inv[:], 127.0, None,
                                    op0=mybir.AluOpType.mult)

            # ---- block-A qw loads (emitted after x so x DMAs win priority)
            BA = 3                      # first mts done kt-outer during chase
            qwA = []
            for mt in range(BA):
                qwt = qwp.tile([128, KT, 128], BF16, tag="qw", name=f"qwA{mt}")
                nc.sync.dma_start(out=qwt[:], in_=qw_d[mt, :, :, :])
                qwA.append(qwt)

            # ---- quantize: x*inv -> int8 (rint+sat) -> bf16; split ACT/DVE
            qxts = []
            for kt in range(KT):
                xs = xch[kt // CH][:, kt % CH, :]
                qi = qip.tile([128, N_SH], I8, tag="qi")
                qx = qxp.tile([128, N_SH], BF16, tag=f"qx{kt}")
                if kt % 2 == 0:
                    nc.scalar.mul(qi[:], xs, inv[:, 0:1])
                    nc.vector.tensor_copy(qx[:], qi[:])
                else:
                    nc.vector.tensor_scalar(qi[:], xs, inv[:, 0:1],
                                            None, op0=mybir.AluOpType.mult)
                    nc.scalar.copy(qx[:], qi[:])
                qxts.append(qx)

            def requant(mt, pss, yo):
                for nh in range(NH):
                    yi = yip.tile([128, 512], I8, tag="yi",
                                  name=f"yi{mt}_{nh}")
                    nc.scalar.activation(yi[:], pss[nh][:],
                                         mybir.ActivationFunctionType.Identity,
                                         bias=b_f32[:, mt:mt + 1],
                                         scale=al_sb[:, 0:1])
                    nc.vector.tensor_scalar(yo[:, nh * 512:(nh + 1) * 512],
                                            yi[:], ys_sb[:, 0:1], None,
                                            op0=mybir.AluOpType.mult)
                nc.sync.dma_start(out=out_d[mt * 128:(mt + 1) * 128, :],
                                  in_=yo[:])

            # ---- block A: kt-outer over first BA mts — PE saturates while
            # quantize streams kts (6 psum banks live)
            psA = {}
            for mt in range(BA):
                for nh in range(NH):
                    ps = psp.tile([128, 512], F32, tag="ps",
                                  name=f"psA{mt}_{nh}")
                    psA[(mt, nh)] = ps
            for kt in range(KT):
                for mt in range(BA):
                    for nh in range(NH):
                        nc.tensor.matmul(
                            psA[(mt, nh)][:],
                            lhsT=qwA[mt][:, kt, :],
                            rhs=qxts[kt][:, nh * 512:(nh + 1) * 512],
                            start=(kt == 0), stop=(kt == KT - 1))
            for mt in range(BA):
                yo = yop.tile([128, N_SH], F16, tag="yo", name=f"yoA{mt}")
                requant(mt, [psA[(mt, nh)] for nh in range(NH)], yo)

            # ---- main loop: remaining mts, mt-outer
            for mt in range(BA, MT):
                qw_t = qwp.tile([128, KT, 128], BF16, tag="qw")
                nc.sync.dma_start(out=qw_t[:], in_=qw_d[mt, :, :, :])
                pss = []
                for nh in range(NH):
                    ps = psp.tile([128, 512], F32, tag="ps")
                    pss.append(ps)
                for kt in range(KT):
                    for nh in range(NH):
                        nc.tensor.matmul(
                            pss[nh][:],
                            lhsT=qw_t[:, kt, :],
                            rhs=qxts[kt][:, nh * 512:(nh + 1) * 512],
                            start=(kt == 0), stop=(kt == KT - 1))
                yo = yop.tile([128, N_SH], F16, tag="yo")
                requant(mt, pss, yo)
    nc.compile()
    return nc


def kernel(x, qw, b_i8, alpha, y_scale):
    global LAST_EXEC_NS
    x = np.asarray(x, dtype=np.float16)
    qw = np.asarray(qw, dtype=np.int8)
    b_i8 = np.asarray(b_i8, dtype=np.int8)
    alpha = np.float32(np.asarray(alpha))
    y_scale = np.float32(np.asarray(y_scale))

    # host-side layout prep (no reference math happens here)
    W = qw.astype(ml_dtypes.bfloat16).reshape(MT, 128, KT, 128)
    W = np.ascontiguousarray(W.transpose(0, 3, 2, 1))        # [mt, p, kt, m]
    B = np.ascontiguousarray(b_i8.reshape(MT, 128).T)        # [p, mt]
    AL = np.full((128, 1), alpha, np.float32)
    YS = np.full((128, 1), y_scale, np.float32)

    in_maps = []
    for i in range(N_CORES):
        xTi = np.ascontiguousarray(x[i * N_SH:(i + 1) * N_SH, :].T)
        in_maps.append({"xT": xTi, "qwt": W, "bt": B,
                        "alpha_b": AL, "ys_b": YS})

    if "nc" not in _nc_cache:
        _nc_cache["nc"] = _build()
    nc = _nc_cache["nc"]

    want_trace = os.environ.get("KERNEL_TRACE", "1") == "1"
    if want_trace:
        _ensure_ntff_hook()
    trace_cores = (list(range(N_CORES))
                   if os.environ.get("KERNEL_TRACE_ALL") == "1" else None)
    try:
        res = run_bass_kernel_spmd(nc, in_maps, list(range(N_CORES)),
                                   trace=want_trace, trace_cores=trace_cores,
                                   tmpdir=os.environ.get("KERNEL_TRACE_DIR"))
    except Exception:
        if not want_trace:
            raise
        res = run_bass_kernel_spmd(nc, in_maps, list(range(N_CORES)),
                                   trace=False)
    LAST_EXEC_NS = res.exec_time_ns

    yT = np.concatenate([res.results[i]["out"] for i in range(N_CORES)],
                        axis=1)                              # [D_OUT, N_TOK]
    return np.ascontiguousarray(yT.T)                        # [N_TOK, D_OUT] f16


# revision 9
# speedup vs baseline: 1.0248x; 1.0248x over previous
"""AdaQLinear (W8A8 dynamic-act-quant linear) on 8 TRN2 NeuronCores.

  reference:
    x_scale = max(|x|)/127                      (global, dynamic)
    qx = clip(round(x/x_scale), -128, 127)      int8
    acc = qx @ qw.T                             int32
    y_i8 = clip(round(acc*alpha) + b_i8, -128, 127)
    y = (y_i8 * y_scale).fp16

  Sharding: data-parallel over tokens (8192/8 = 1024 per core); qw/b/scales
  replicated.  Each core needs the *global* absmax of x, obtained with a
  tiny AllReduce(max) of per-partition maxima.

  The int8 GEMM runs as bf16 matmul: qx,qw in [-128,127] are exact in bf16
  and PSUM accumulates fp32 (sums stay far below 2^24), so results match
  the int32 reference bit-exactly.  Requant = one ACT op per tile
  (rint(acc*alpha + b) with saturating int8 cast == round+add+clip since b
  is integral) + one DVE scale-mul to fp16.
"""
import os
import numpy as np
import ml_dtypes

import concourse.bass as bass
import concourse.bass_isa as bass_isa
import concourse.tile as tile
from concourse import bacc, mybir
import concourse.bass_utils as bass_utils
from concourse.bass_utils import run_bass_kernel_spmd

# Don't ship trace artifacts anywhere external; keep them local.
bass_utils.upload_artifacts = lambda tmpdir: ""


def _ensure_ntff_hook():
    """Register the axon NTFF profiling hook if the image's antenv lacks
    the axon_hooks shim (needed for exec_time_ns with trace=True)."""
    try:
        from antenv.axon_hooks import get_axon_ntff_profile_hook  # noqa: F401
        return
    except ImportError:
        pass
    import sys
    import types
    hook = None
    try:
        from trn_agent_boot.trn_boot import _ntff_profile_via_ctypes
        so = "/opt/axon/libaxon_pjrt.so"
        if os.path.exists(so):
            hook = _ntff_profile_via_ctypes(so)
    except Exception:
        hook = None
    mod = types.ModuleType("antenv.axon_hooks")
    mod._hook = hook
    mod.get_axon_ntff_profile_hook = lambda: mod._hook
    mod.set_axon_ntff_profile_hook = lambda h: setattr(mod, "_hook", h)
    sys.modules["antenv.axon_hooks"] = mod
    try:
        import antenv
        antenv.axon_hooks = mod
    except Exception:
        pass

N_TOK, D_IN, D_OUT = 8192, 4096, 4096
N_CORES = 8
N_SH = N_TOK // N_CORES      # 1024 tokens per core
KT = D_IN // 128             # 32 contraction tiles
MT = D_OUT // 128            # 32 output-row tiles
NH = N_SH // 512             # 2 psum-width halves

F16, BF16, F32 = mybir.dt.float16, mybir.dt.bfloat16, mybir.dt.float32
I8 = mybir.dt.int8

_nc_cache = {}
LAST_EXEC_NS = None


def _build():
    nc = bacc.Bacc("TRN2", target_bir_lowering=False, debug=False,
                   num_devices=N_CORES)
    xT_d = nc.dram_tensor("xT", [D_IN, N_SH], F16, kind="ExternalInput")
    qw_d = nc.dram_tensor("qwt", [MT, 128, KT, 128], BF16, kind="ExternalInput")
    b_d = nc.dram_tensor("bt", [128, MT], I8, kind="ExternalInput")
    al_d = nc.dram_tensor("alpha_b", [128, 1], F32, kind="ExternalInput")
    ys_d = nc.dram_tensor("ys_b", [128, 1], F32, kind="ExternalInput")
    out_d = nc.dram_tensor("out", [D_OUT, N_SH], F16, kind="ExternalOutput")
    cc_out = nc.dram_tensor("cc_out", [128, 1], F32, addr_space="Shared")

    with tile.TileContext(nc) as tc:
        with tc.tile_pool(name="const", bufs=1) as cp, \
             tc.tile_pool(name="xin", bufs=1) as xp, \
             tc.tile_pool(name="qx", bufs=1) as qxp, \
             tc.tile_pool(name="qi", bufs=4) as qip, \
             tc.tile_pool(name="ax", bufs=2) as axp, \
             tc.tile_pool(name="qw", bufs=5) as qwp, \
             tc.tile_pool(name="ps", bufs=6, space="PSUM") as psp, \
             tc.tile_pool(name="yi", bufs=4) as yip, \
             tc.tile_pool(name="yo", bufs=3) as yop, \
             tc.tile_pool(name="dram", bufs=1, space="DRAM") as dp:

            # ---- constants
            b_i8 = cp.tile([128, MT], I8, tag="bi8")
            nc.sync.dma_start(out=b_i8[:], in_=b_d[:])
            b_f32 = cp.tile([128, MT], F32, tag="bf32")
            nc.vector.tensor_copy(b_f32[:], b_i8[:])
            al_sb = cp.tile([128, 1], F32, tag="al")
            nc.sync.dma_start(out=al_sb[:], in_=al_d[:])
            ys_sb = cp.tile([128, 1], F32, tag="ys")
            nc.sync.dma_start(out=ys_sb[:], in_=ys_d[:])

            # ---- load x shard (2kt chunks); absmax (u16-mask abs, TT max)
            CH = 2                      # kt per chunk
            xch = []
            acc = cp.tile([128, CH * N_SH], F16, tag="amacc")
            nc.vector.memset(acc[:], 0.0)
            for c in range(KT // CH):
                xt = xp.tile([128, CH, N_SH], F16, tag=f"x{c}")
                nc.sync.dma_start(
                    out=xt[:],
                    in_=xT_d[c * CH * 128:(c + 1) * CH * 128, :].rearrange(
                        "(k p) n -> p k n", p=128))
                xch.append(xt)
                ax = axp.tile([128, CH * N_SH], F16, tag="ax")
                nc.vector.tensor_scalar(ax[:].bitcast(mybir.dt.uint16),
                                        xt[:].opt().bitcast(mybir.dt.uint16),
                                        0x7FFF, None,
                                        op0=mybir.AluOpType.bitwise_and)
                nc.vector.tensor_tensor(acc[:], acc[:], ax[:],
                                        op=mybir.AluOpType.max)
            am = cp.tile([128, 1], F32, tag="am")
            nc.vector.tensor_reduce(am[:], acc[:], axis=mybir.AxisListType.X,
                                    op=mybir.AluOpType.max,
                                    apply_absolute_value=True)

            # ---- global absmax: cross-partition max first (hides under the
            # ncfw boot window), then AllReduce(max) across cores
            lm = cp.tile([128, 1], F32, tag="lm")
            nc.gpsimd.partition_all_reduce(lm[:], am[:], 128,
                                           bass_isa.ReduceOp.max)
            cc_in = dp.tile([128, 1], F32)
            nc.sync.dma_start(out=cc_in[:], in_=lm[:])
            nc.gpsimd.collective_compute(
                "AllReduce", mybir.AluOpType.max,
                replica_groups=[list(range(N_CORES))],
                ins=[cc_in[:].opt()], outs=[cc_out.ap().opt()])
            gm = cp.tile([128, 1], F32, tag="gm")
            nc.sync.dma_start(out=gm[:], in_=cc_out[:, :])
            inv = cp.tile([128, 1], F32, tag="inv")
            nc.vector.reciprocal(inv[:], gm[:])
            nc.vector.tensor_scalar(inv[:], inv[:], 127.0, None,
                                    op0=mybir.AluOpType.mult)

            # ---- block-A qw loads (emitted after x so x DMAs win priority)
            BA = 3                      # first mts done kt-outer during chase
            qwA = []
            for mt in range(BA):
                qwt = qwp.tile([128, KT, 128], BF16, tag="qw", name=f"qwA{mt}")
                nc.sync.dma_start(out=qwt[:], in_=qw_d[mt, :, :, :])
                qwA.append(qwt)

            # ---- quantize: x*inv -> int8 (rint+sat) -> bf16; split ACT/DVE
            qxts = []
            for kt in range(KT):
                xs = xch[kt // CH][:, kt % CH, :]
                qi = qip.tile([128, N_SH], I8, tag="qi")
                qx = qxp.tile([128, N_SH], BF16, tag=f"qx{kt}")
                if kt % 2 == 0:
                    nc.scalar.mul(qi[:], xs, inv[:, 0:1])
                    nc.vector.tensor_copy(qx[:], qi[:])
                else:
                    nc.vector.tensor_scalar(qi[:], xs, inv[:, 0:1],
                                            None, op0=mybir.AluOpType.mult)
                    nc.scalar.copy(qx[:], qi[:])
                qxts.append(qx)

            def requant(mt, pss, yo):
                for nh in range(NH):
                    yi = yip.tile([128, 512], I8, tag="yi",
                                  name=f"yi{mt}_{nh}")
                    nc.scalar.activation(yi[:], pss[nh][:],
                                         mybir.ActivationFunctionType.Identity,
                                         bias=b_f32[:, mt:mt + 1],
                                         scale=al_sb[:, 0:1])
                    nc.vector.tensor_scalar(yo[:, nh * 512:(nh + 1) * 512],
                                            yi[:], ys_sb[:, 0:1], None,
                                            op0=mybir.AluOpType.mult)
                nc.sync.dma_start(out=out_d[mt * 128:(mt + 1) * 128, :],
                                  in_=yo[:])

            # ---- block A: kt-outer over first BA mts — PE saturates while
            # quantize streams kts (6 psum banks live)
            psA = {}
            for mt in range(BA):
                for nh in range(NH):
                    ps = psp.tile([128, 512], F32, tag="ps",
                                  name=f"psA{mt}_{nh}")
                    psA[(mt, nh)] = ps
            for kt in range(KT):
                for mt in range(BA):
                    for nh in range(NH):
                        nc.tensor.matmul(
                            psA[(mt, nh)][:],
                            lhsT=qwA[mt][:, kt, :],
                            rhs=qxts[kt][:, nh * 512:(nh + 1) * 512],
                            start=(kt == 0), stop=(kt == KT - 1))
            for mt in range(BA):
                yo = yop.tile([128, N_SH], F16, tag="yo", name=f"yoA{mt}")
                requant(mt, [psA[(mt, nh)] for nh in range(NH)], yo)

            # ---- main loop: remaining mts, mt-outer
            for mt in range(BA, MT):
                qw_t = qwp.tile([128, KT, 128], BF16, tag="qw")
                nc.sync.dma_start(out=qw_t[:], in_=qw_d[mt, :, :, :])
                pss = []
                for nh in range(NH):
                    ps = psp.tile([128, 512], F32, tag="ps")
                    pss.append(ps)
                for kt in range(KT):
                    for nh in range(NH):
                        nc.tensor.matmul(
                            pss[nh][:],
                            lhsT=qw_t[:, kt, :],
                            rhs=qxts[kt][:, nh * 512:(nh + 1) * 512],
                            start=(kt == 0), stop=(kt == KT - 1))
                yo = yop.tile([128, N_SH], F16, tag="yo")
                requant(mt, pss, yo)
    nc.compile()
    return nc


def kernel(x, qw, b_i8, alpha, y_scale):
    global LAST_EXEC_NS
    x = np.asarray(x, dtype=np.float16)
    qw = np.asarray(qw, dtype=np.int8)
    b_i8 = np.asarray(b_i8, dtype=np.int8)
    alpha = np.float32(np.asarray(alpha))
    y_scale = np.float32(np.asarray(y_scale))

    # host-side layout prep (no reference math happens here)
    W = qw.astype(ml_dtypes.bfloat16).reshape(MT, 128, KT, 128)
    W = np.ascontiguousarray(W.transpose(0, 3, 2, 1))        # [mt, p, kt, m]
    B = np.ascontiguousarray(b_i8.reshape(MT, 128).T)        # [p, mt]
    AL = np.full((128, 1), alpha, np.float32)
    YS = np.full((128, 1), y_scale, np.float32)

    in_maps = []
    for i in range(N_CORES):
        xTi = np.ascontiguousarray(x[i * N_SH:(i + 1) * N_SH, :].T)
        in_maps.append({"xT": xTi, "qwt": W, "bt": B,
                        "alpha_b": AL, "ys_b": YS})

    if "nc" not in _nc_cache:
        _nc_cache["nc"] = _build()
    nc = _nc_cache["nc"]

    want_trace = os.environ.get("KERNEL_TRACE", "0") == "1"
    if want_trace:
        _ensure_ntff_hook()
    trace_cores = (list(range(N_CORES))
                   if os.environ.get("KERNEL_TRACE_ALL") == "1" else None)
    try:
        res = run_bass_kernel_spmd(nc, in_maps, list(range(N_CORES)),
                                   trace=want_trace, trace_cores=trace_cores,
                                   tmpdir=os.environ.get("KERNEL_TRACE_DIR"))
    except Exception:
        if not want_trace:
            raise
        res = run_bass_kernel_spmd(nc, in_maps, list(range(N_CORES)),
                                   trace=False)
    LAST_EXEC_NS = res.exec_time_ns

    yT = np.concatenate([res.results[i]["out"] for i in range(N_CORES)],
                        axis=1)                              # [D_OUT, N_TOK]
    return np.ascontiguousarray(yT.T)                        # [N_TOK, D_OUT] f16
